# revision 64
# baseline (speedup 1.0000x reference)
"""MoE (top-2 of 8 experts) Trainium2 kernel.

Strategy: expert-parallel across the 8 NeuronCores. The router
(softmax + top-2 over [T, 8] logits) is metadata computed on host to
build the dispatch; core e receives only the tokens routed to expert e
(gathered, transposed, zero-padded to a common capacity C) plus that
expert's weights, pre-transposed so the device does no transposes:

  core e inputs:  xT  [H, C]   = x[idx_e].T        (padded)
                  w1T [H, I]   = w1[e].T
                  w2T [I, H]   = w2[e].T
                  gates [1, C]  renormalized top-2 weight per token
  core e output:  yT  [H, C]  = (gate * (silu(x_e @ w1[e].T) @ w2[e].T)).T

On device (per core, fp32 storage, float32r matmuls, only the exact
even-rounded token count is computed — no padding columns):
  stage 1: hT[i_tile, c_chunk] = silu(w1T.T @ xT)    (I on partitions)
  stage 2: yT[h_tile, c_chunk] = w2T.T @ hT, times the per-token gate
           (w2 stationary, hT moving: the stream covers the ragged token
           dim; gate is broadcast to all partitions by a 0-stride DMA)

The host transposes and scatter-adds the two expert contributions per
token.
"""

import numpy as np

import concourse.mybir as mybir
from concourse import bacc
from concourse.tile import TileContext
from concourse.bass_utils import run_bass_kernel_spmd

T, H, I, E = 4096, 1024, 1408, 8
TOPK = 2
P = 128
CHUNK = 512
N_CORES = 8
F32 = mybir.dt.float32
F32R = mybir.dt.float32r
AF = mybir.ActivationFunctionType

# most recently built device program (for test harnesses / cost-model timing)
LAST_NC = None


def _chunk_sizes(C):
    """Split C into ceil(C/512) chunks, multiples of 128, as even as
    possible. Balanced chunks keep every stage-1 matmul's moving dim >=256
    (the fp32r full-rate threshold) instead of a slow ragged tail."""
    n = -(-C // CHUNK)
    base = (C // n) // P * P
    rem = (C - n * base) // P
    return [base + P if j < rem else base for j in range(n)]


def _count_chunks(count):
    """Chunk an exact token count: every chunk starts 128-aligned, all but
    the last are multiples of 128, the last is ragged — no cycles are spent
    on padding columns. Balanced so chunks stay >=256 wide when possible."""
    n = -(-count // CHUNK)
    tiles = -(-count // P)
    per, rem = divmod(tiles, n)
    tile_counts = [per + 1 if j < rem else per for j in range(n)]
    sizes = [tc * P for tc in tile_counts]
    sizes[-1] -= tiles * P - count
    return sizes


def build_moe_expert_kernel(count, h=H, i_dim=I):
    """One-expert MLP over `count` gathered tokens (any positive int —
    DRAM buffers are padded to a 128 multiple, but only `count` columns
    are computed). h, i_dim overridable for small-scale simulation tests;
    both must be multiples of 128. count must be even (fp32r matmuls
    reject odd free/partition sizes)."""
    C = -(-count // P) * P  # DRAM/layout capacity
    assert count % 2 == 0 and h % P == 0 and i_dim % P == 0
    HK = h // P
    IT = i_dim // P

    nc = bacc.Bacc("TRN2", target_bir_lowering=False, debug=False, num_devices=N_CORES)
    # Matmul inputs are stored as float32r (same 32-bit layout; the PE
    # rounds to its reduced internal precision). Typing the whole producer
    # chain as f32r satisfies the BIR verifier's rounding check.
    xT = nc.dram_tensor("xT", [h, C], F32R, kind="ExternalInput").ap()
    w1T = nc.dram_tensor("w1T", [h, i_dim], F32R, kind="ExternalInput").ap()
    w2T = nc.dram_tensor("w2T", [i_dim, h], F32R, kind="ExternalInput").ap()
    gates = nc.dram_tensor("gates", [1, C], F32, kind="ExternalInput").ap()
    # output is yT [h, C]: stage 2 streams over the ragged token dim, so
    # tokens land on the free axis (the host transposes back)
    yT = nc.dram_tensor("yT", [h, C], F32, kind="ExternalOutput").ap()

    xT_v = xT.rearrange("(ho p) c -> p ho c", p=P)  # [128, HK, C]
    w1T_v = w1T.rearrange("(ho p) i -> p ho i", p=P)  # [128, HK, I]
    w2T_v = w2T.rearrange("(io p) h -> p io h", p=P)  # [128, IT, H]
    yT_v = yT.rearrange("(ho p) c -> ho p c", p=P)  # [HK, 128, C]

    h_chunks = _chunk_sizes(h)  # h-chunks for stage 2 output
    c_chunks = _count_chunks(count)
    max_cs = max(c_chunks)
    c_starts = [sum(c_chunks[:j]) for j in range(len(c_chunks))]
    # per-partition SBUF bytes: weights + broadcast gates + h/sg bufs; give
    # the x and y pools extra bufs only while the 192 KB budget holds
    base = 4 * (HK * i_dim + IT * h + C + 2 * IT * max_cs + 2 * CHUNK)
    x_bufs = 3 if base + 3 * 4 * HK * max_cs + 2 * 4 * CHUNK < 190 * 1024 else 2
    fixed = base + x_bufs * 4 * HK * max_cs
    y_bufs = 4 if fixed + 4 * 4 * CHUNK < 190 * 1024 else 2
    with TileContext(nc) as tc:
        with (
            tc.tile_pool(name="wpool", bufs=1) as wpool,
            tc.tile_pool(name="xpool", bufs=x_bufs) as xpool,
            tc.tile_pool(name="hpool", bufs=2) as hpool,
            tc.tile_pool(name="ypool", bufs=y_bufs) as ypool,
            tc.tile_pool(name="sgpool", bufs=2) as sgpool,
            tc.tile_pool(name="ps1", bufs=4, space="PSUM") as ps1pool,
            tc.tile_pool(name="ps2", bufs=4, space="PSUM") as ps2pool,
        ):
            # per-token gate replicated to all partitions: one DMA reading
            # the same DRAM row 128x (0-stride partition source)
            gb = wpool.tile([P, C], F32)
            w1s = wpool.tile([P, HK, i_dim], F32R)
            w2s = wpool.tile([P, IT, h], F32R)
            xs_tiles = {}

            def load_x(ci, split=True):
                # per-hk DMAs deliver the chunk incrementally so stage-1
                # groups can start before the whole chunk lands
                xs = xpool.tile([P, HK, max_cs], F32R, tag="xs", name=f"xs{ci}")
                cs, c0 = c_chunks[ci], c_starts[ci]
                if split:
                    for hk in range(HK):
                        nc.sync.dma_start(xs[:, hk, :cs], xT_v[:, hk, c0 : c0 + cs])
                else:
                    nc.sync.dma_start(xs[:, :, :cs], xT_v[:, :, c0 : c0 + cs])
                xs_tiles[ci] = xs

            def load_w1(it):
                nc.sync.dma_start(
                    w1s[:, :, it * P : (it + 1) * P],
                    w1T_v[:, :, it * P : (it + 1) * P],
                )

            # DMA issue order = consumption order. Interleave chunk-0 x
            # slices with the leading w1 i-tiles so the first stage-1
            # accumulation group starts after ~0.7 MB instead of ~6 MB;
            # then the rest of w1, the remaining x chunks, then w2 (per
            # h-half, consumed by stage 2).
            xs0 = xpool.tile([P, HK, max_cs], F32R, tag="xs", name="xs0")
            cs0 = c_chunks[0]
            load_w1(0)
            for hk in range(HK):
                nc.sync.dma_start(xs0[:, hk, :cs0], xT_v[:, hk, 0:cs0])
                if hk == 1:
                    load_w1(1)
            xs_tiles[0] = xs0
            for it in range(2, IT):
                load_w1(it)
            # w2 per h-half per i-tile: stage 2 consumes one h-chunk across
            # i-tiles in order, so fine-grained delivery unblocks each
            # accumulation group as early as possible
            h_starts = [sum(h_chunks[:j]) for j in range(len(h_chunks))]
            # only as many x chunks up front as there are pool slots — a
            # queued DMA waiting on a busy slot would head-of-line block
            # the w2 stream behind it; later chunks prefetch inside stage 1
            for ci in range(1, min(x_bufs, len(c_chunks))):
                load_x(ci)
            # broadcast-gate load sits after the stage-1 streams (it is
            # only needed when the first stage-2 group's psum is evacuated)
            nc.sync.dma_start(gb[:], gates[0].partition_broadcast(P))
            for h0, hcs in zip(h_starts, h_chunks):
                for it in range(IT):
                    nc.sync.dma_start(
                        w2s[:, it, h0 : h0 + hcs], w2T_v[:, it, h0 : h0 + hcs]
                    )

            hs_tiles = {}

            def stage1(ci):
                nxt = ci + 1
                if nxt < len(c_chunks) and nxt not in xs_tiles:
                    load_x(nxt)
                cs = c_chunks[ci]
                xs = xs_tiles[ci]
                # hT = silu(w1T.T @ xT)  -> [I, cs], I on partitions
                hs = hpool.tile([P, IT, max_cs], F32R, tag="hs", name=f"hs{ci}")
                for it in range(IT):
                    ps1 = ps1pool.tile([P, CHUNK], F32, tag="ps1")
                    for hk in range(HK):
                        nc.tensor.matmul(
                            ps1[:, :cs],
                            w1s[:, hk, it * P : (it + 1) * P],
                            xs[:, hk, :cs],
                            start=(hk == 0),
                            stop=(hk == HK - 1),
                        )
                    # silu(z) = z * sigmoid(z); CoreSim has no Silu table,
                    # so build it from Sigmoid (ACT) + multiply (DVE)
                    sg = sgpool.tile([P, CHUNK], F32, tag="sg")
                    nc.scalar.activation(sg[:, :cs], ps1[:, :cs], AF.Sigmoid)
                    nc.vector.tensor_mul(
                        out=hs[:, it, :cs], in0=ps1[:, :cs], in1=sg[:, :cs]
                    )
                hs_tiles[ci] = hs

            def stage2(ci):
                # yT = (w2T.T @ hT) * gate -> [H, cs], h on partitions.
                # w2 is the stationary operand and hT the moving one, so the
                # stream covers exactly the ragged token count — no padded
                # columns and no partial-partition tiles.
                cs, c0 = c_chunks[ci], c_starts[ci]
                hs = hs_tiles.pop(ci)
                for ht in range(HK):
                    ps2 = ps2pool.tile([P, CHUNK], F32, tag="ps2")
                    for it in range(IT):
                        nc.tensor.matmul(
                            ps2[:, :cs],
                            w2s[:, it, ht * P : (ht + 1) * P],
                            hs[:, it, :cs],
                            start=(it == 0),
                            stop=(it == IT - 1),
                        )
                    ys = ypool.tile([P, CHUNK], F32, tag="ys")
                    nc.vector.tensor_mul(
                        out=ys[:, :cs], in0=ps2[:, :cs], in1=gb[:, c0 : c0 + cs]
                    )
                    nc.sync.dma_start(yT_v[ht][:, c0 : c0 + cs], ys[:, :cs])

            # software pipeline: run stage 1 a chunk ahead so the PE has
            # stage-1 work for chunk i+1 while w2 is still streaming in
            stage1(0)
            for ci in range(1, len(c_chunks)):
                stage1(ci)
                stage2(ci - 1)
            stage2(len(c_chunks) - 1)
    nc.compile()
    global LAST_NC
    LAST_NC = nc
    return nc


def route(router_logits):
    """Host-side router: softmax -> top-2 -> renormalize.

    Returns (top2_idx [T,2] int64, top2_gate [T,2] float32)."""
    logits = np.asarray(router_logits, dtype=np.float32)
    m = logits.max(axis=-1, keepdims=True)
    ex = np.exp(logits - m)
    probs = ex / ex.sum(axis=-1, keepdims=True)
    order = np.argsort(-probs, axis=-1, kind="stable")[:, :TOPK]
    rows = np.arange(logits.shape[0])[:, None]
    topk_p = probs[rows, order]
    topk_p = topk_p / topk_p.sum(axis=-1, keepdims=True)
    return order, topk_p.astype(np.float32)


def kernel(x, router_logits, w1, w2):
    x = np.ascontiguousarray(np.asarray(x, dtype=np.float32))
    w1 = np.asarray(w1, dtype=np.float32)
    w2 = np.asarray(w2, dtype=np.float32)
    t = x.shape[0]

    top2_idx, top2_gate = route(router_logits)

    expert_tokens = []
    expert_gates = []
    for e in range(E):
        sel = np.nonzero(top2_idx == e)
        expert_tokens.append(sel[0])
        expert_gates.append(top2_gate[sel[0], sel[1]])
    counts = [len(ix) for ix in expert_tokens]
    # fp32r matmuls require even free/partition sizes (2-element PSUM
    # interleave), so round the computed token count up to even
    count = max(2, max(counts) + max(counts) % 2)
    C = -(-count // P) * P  # buffer capacity (128-aligned)

    nc = build_moe_expert_kernel(count)

    in_maps = []
    for e in range(E):
        cnt = counts[e]
        xT_e = np.zeros((H, C), dtype=np.float32)
        xT_e[:, :cnt] = x[expert_tokens[e]].T
        g = np.zeros((1, C), dtype=np.float32)
        g[0, :cnt] = expert_gates[e]
        in_maps.append(
            {
                "xT": xT_e,
                "w1T": np.ascontiguousarray(w1[e].T),
                "w2T": np.ascontiguousarray(w2[e].T),
                "gates": g,
            }
        )

    res = run_bass_kernel_spmd(nc, in_maps, core_ids=list(range(N_CORES)))
    if not all(np.isfinite(r["yT"]).all() for r in res.results):
        # one retry in case of a transient device fault
        res = run_bass_kernel_spmd(nc, in_maps, core_ids=list(range(N_CORES)))

    out = np.zeros((t, H), dtype=np.float32)
    for e in range(E):
        cnt = counts[e]
        out[expert_tokens[e]] += res.results[e]["yT"][:, :cnt].T
    return out


# revision 67
# speedup vs baseline: 1.0044x; 1.0044x over previous
"""MoE (top-2 of 8 experts) Trainium2 kernel.

Strategy: expert-parallel across the 8 NeuronCores. The router
(softmax + top-2 over [T, 8] logits) is metadata computed on host to
build the dispatch; core e receives only the tokens routed to expert e
(gathered, transposed, zero-padded to a common capacity C) plus that
expert's weights, pre-transposed so the device does no transposes:

  core e inputs:  xT  [H, C]   = x[idx_e].T        (padded)
                  w1T [H, I]   = w1[e].T
                  w2T [I, H]   = w2[e].T
                  gates [1, C]  renormalized top-2 weight per token
  core e output:  yT  [H, C]  = (gate * (silu(x_e @ w1[e].T) @ w2[e].T)).T

On device (per core, fp32 storage, float32r matmuls, only the exact
even-rounded token count is computed — no padding columns):
  stage 1: hT[i_tile, c_chunk] = silu(w1T.T @ xT)    (I on partitions)
  stage 2: yT[h_tile, c_chunk] = w2T.T @ hT, times the per-token gate
           (w2 stationary, hT moving: the stream covers the ragged token
           dim; gate is broadcast to all partitions by a 0-stride DMA)

The host transposes and scatter-adds the two expert contributions per
token.
"""

import numpy as np

import concourse.mybir as mybir
from concourse import bacc
from concourse.tile import TileContext
from concourse.bass_utils import run_bass_kernel_spmd

T, H, I, E = 4096, 1024, 1408, 8
TOPK = 2
P = 128
CHUNK = 512
N_CORES = 8
F32 = mybir.dt.float32
F32R = mybir.dt.float32r
AF = mybir.ActivationFunctionType

# most recently built device program (for test harnesses / cost-model timing)
LAST_NC = None


def _chunk_sizes(C):
    """Split C into ceil(C/512) chunks, multiples of 128, as even as
    possible. Balanced chunks keep every stage-1 matmul's moving dim >=256
    (the fp32r full-rate threshold) instead of a slow ragged tail."""
    n = -(-C // CHUNK)
    base = (C // n) // P * P
    rem = (C - n * base) // P
    return [base + P if j < rem else base for j in range(n)]


def _count_chunks(count):
    """Chunk an exact token count: every chunk starts 128-aligned, all but
    the last are multiples of 128, the last is ragged — no cycles are spent
    on padding columns. Front chunks are full 512 so stage-1 groups outlast
    the w1 tile arrival period (no DMA-pacing stalls during the weight
    stream); the tail is split to keep every chunk >=256 (the fp32r
    full-rate threshold) whenever count allows."""
    full, rem = divmod(count, CHUNK)
    if rem == 0:
        return [CHUNK] * full
    if rem >= 256 or full == 0:
        return [CHUNK] * full + [rem]
    # rem < 256: borrow from one full chunk so both tail chunks stay >=256
    return [CHUNK] * (full - 1) + [256, 256 + rem]


def build_moe_expert_kernel(count, h=H, i_dim=I):
    """One-expert MLP over `count` gathered tokens (any positive int —
    DRAM buffers are padded to a 128 multiple, but only `count` columns
    are computed). h, i_dim overridable for small-scale simulation tests;
    both must be multiples of 128. count must be even (fp32r matmuls
    reject odd free/partition sizes)."""
    C = -(-count // P) * P  # DRAM/layout capacity
    assert count % 2 == 0 and h % P == 0 and i_dim % P == 0
    HK = h // P
    IT = i_dim // P

    nc = bacc.Bacc("TRN2", target_bir_lowering=False, debug=False, num_devices=N_CORES)
    # Matmul inputs are stored as float32r (same 32-bit layout; the PE
    # rounds to its reduced internal precision). Typing the whole producer
    # chain as f32r satisfies the BIR verifier's rounding check.
    xT = nc.dram_tensor("xT", [h, C], F32R, kind="ExternalInput").ap()
    w1T = nc.dram_tensor("w1T", [h, i_dim], F32R, kind="ExternalInput").ap()
    w2T = nc.dram_tensor("w2T", [i_dim, h], F32R, kind="ExternalInput").ap()
    gates = nc.dram_tensor("gates", [1, C], F32, kind="ExternalInput").ap()
    # output is yT [h, C]: stage 2 streams over the ragged token dim, so
    # tokens land on the free axis (the host transposes back)
    yT = nc.dram_tensor("yT", [h, C], F32, kind="ExternalOutput").ap()

    xT_v = xT.rearrange("(ho p) c -> p ho c", p=P)  # [128, HK, C]
    w1T_v = w1T.rearrange("(ho p) i -> p ho i", p=P)  # [128, HK, I]
    w2T_v = w2T.rearrange("(io p) h -> p io h", p=P)  # [128, IT, H]
    yT_v = yT.rearrange("(ho p) c -> ho p c", p=P)  # [HK, 128, C]

    h_chunks = _chunk_sizes(h)  # h-chunks for stage 2 output
    c_chunks = _count_chunks(count)
    max_cs = max(c_chunks)
    c_starts = [sum(c_chunks[:j]) for j in range(len(c_chunks))]
    # per-partition SBUF bytes: weights + broadcast gates + h/sg bufs; give
    # the x and y pools extra bufs only while the 192 KB budget holds
    base = 4 * (HK * i_dim + IT * h + C + 2 * IT * max_cs + 2 * CHUNK)
    x_bufs = 3 if base + 3 * 4 * HK * max_cs + 2 * 4 * CHUNK < 190 * 1024 else 2
    fixed = base + x_bufs * 4 * HK * max_cs
    y_bufs = 4 if fixed + 4 * 4 * CHUNK < 190 * 1024 else 2
    with TileContext(nc) as tc:
        with (
            tc.tile_pool(name="wpool", bufs=1) as wpool,
            tc.tile_pool(name="xpool", bufs=x_bufs) as xpool,
            tc.tile_pool(name="hpool", bufs=2) as hpool,
            tc.tile_pool(name="ypool", bufs=y_bufs) as ypool,
            tc.tile_pool(name="sgpool", bufs=2) as sgpool,
            tc.tile_pool(name="ps1", bufs=4, space="PSUM") as ps1pool,
            tc.tile_pool(name="ps2", bufs=4, space="PSUM") as ps2pool,
        ):
            # per-token gate replicated to all partitions: one DMA reading
            # the same DRAM row 128x (0-stride partition source)
            gb = wpool.tile([P, C], F32)
            w1s = wpool.tile([P, HK, i_dim], F32R)
            w2s = wpool.tile([P, IT, h], F32R)
            xs_tiles = {}

            def load_x(ci, split=True):
                # per-hk DMAs deliver the chunk incrementally so stage-1
                # groups can start before the whole chunk lands
                xs = xpool.tile([P, HK, max_cs], F32R, tag="xs", name=f"xs{ci}")
                cs, c0 = c_chunks[ci], c_starts[ci]
                if split:
                    for hk in range(HK):
                        nc.sync.dma_start(xs[:, hk, :cs], xT_v[:, hk, c0 : c0 + cs])
                else:
                    nc.sync.dma_start(xs[:, :, :cs], xT_v[:, :, c0 : c0 + cs])
                xs_tiles[ci] = xs

            def load_w1(it):
                nc.sync.dma_start(
                    w1s[:, :, it * P : (it + 1) * P],
                    w1T_v[:, :, it * P : (it + 1) * P],
                )

            # DMA issue order = consumption order. Interleave chunk-0 x
            # slices with the leading w1 i-tiles so the first stage-1
            # accumulation group starts after ~0.7 MB instead of ~6 MB;
            # then the rest of w1, the remaining x chunks, then w2 (per
            # h-half, consumed by stage 2).
            xs0 = xpool.tile([P, HK, max_cs], F32R, tag="xs", name="xs0")
            cs0 = c_chunks[0]
            load_w1(0)
            for hk in range(HK):
                nc.sync.dma_start(xs0[:, hk, :cs0], xT_v[:, hk, 0:cs0])
                if hk == 1:
                    load_w1(1)
            xs_tiles[0] = xs0
            for it in range(2, IT):
                load_w1(it)
            # w2 per h-half per i-tile: stage 2 consumes one h-chunk across
            # i-tiles in order, so fine-grained delivery unblocks each
            # accumulation group as early as possible
            h_starts = [sum(h_chunks[:j]) for j in range(len(h_chunks))]
            # only as many x chunks up front as there are pool slots — a
            # queued DMA waiting on a busy slot would head-of-line block
            # the w2 stream behind it; later chunks prefetch inside stage 1
            for ci in range(1, min(x_bufs, len(c_chunks))):
                load_x(ci)
            # broadcast-gate load sits after the stage-1 streams (it is
            # only needed when the first stage-2 group's psum is evacuated)
            nc.sync.dma_start(gb[:], gates[0].partition_broadcast(P))
            for h0, hcs in zip(h_starts, h_chunks):
                for it in range(IT):
                    nc.sync.dma_start(
                        w2s[:, it, h0 : h0 + hcs], w2T_v[:, it, h0 : h0 + hcs]
                    )

            hs_tiles = {}

            def stage1(ci):
                nxt = ci + 1
                if nxt < len(c_chunks) and nxt not in xs_tiles:
                    load_x(nxt)
                cs = c_chunks[ci]
                xs = xs_tiles[ci]
                # hT = silu(w1T.T @ xT)  -> [I, cs], I on partitions
                hs = hpool.tile([P, IT, max_cs], F32R, tag="hs", name=f"hs{ci}")
                for it in range(IT):
                    ps1 = ps1pool.tile([P, CHUNK], F32, tag="ps1")
                    for hk in range(HK):
                        nc.tensor.matmul(
                            ps1[:, :cs],
                            w1s[:, hk, it * P : (it + 1) * P],
                            xs[:, hk, :cs],
                            start=(hk == 0),
                            stop=(hk == HK - 1),
                        )
                    # silu(z) = z * sigmoid(z); CoreSim has no Silu table,
                    # so build it from Sigmoid (ACT) + multiply (DVE)
                    sg = sgpool.tile([P, CHUNK], F32, tag="sg")
                    nc.scalar.activation(sg[:, :cs], ps1[:, :cs], AF.Sigmoid)
                    nc.vector.tensor_mul(
                        out=hs[:, it, :cs], in0=ps1[:, :cs], in1=sg[:, :cs]
                    )
                hs_tiles[ci] = hs

            def stage2(ci):
                # yT = (w2T.T @ hT) * gate -> [H, cs], h on partitions.
                # w2 is the stationary operand and hT the moving one, so the
                # stream covers exactly the ragged token count — no padded
                # columns and no partial-partition tiles.
                cs, c0 = c_chunks[ci], c_starts[ci]
                hs = hs_tiles.pop(ci)
                for ht in range(HK):
                    ps2 = ps2pool.tile([P, CHUNK], F32, tag="ps2")
                    for it in range(IT):
                        nc.tensor.matmul(
                            ps2[:, :cs],
                            w2s[:, it, ht * P : (ht + 1) * P],
                            hs[:, it, :cs],
                            start=(it == 0),
                            stop=(it == IT - 1),
                        )
                    ys = ypool.tile([P, CHUNK], F32, tag="ys")
                    nc.vector.tensor_mul(
                        out=ys[:, :cs], in0=ps2[:, :cs], in1=gb[:, c0 : c0 + cs]
                    )
                    nc.sync.dma_start(yT_v[ht][:, c0 : c0 + cs], ys[:, :cs])

            # software pipeline: run stage 1 a chunk ahead so the PE has
            # stage-1 work for chunk i+1 while w2 is still streaming in
            stage1(0)
            for ci in range(1, len(c_chunks)):
                stage1(ci)
                stage2(ci - 1)
            stage2(len(c_chunks) - 1)
    nc.compile()
    global LAST_NC
    LAST_NC = nc
    return nc


def route(router_logits):
    """Host-side router: softmax -> top-2 -> renormalize.

    Returns (top2_idx [T,2] int64, top2_gate [T,2] float32)."""
    logits = np.asarray(router_logits, dtype=np.float32)
    m = logits.max(axis=-1, keepdims=True)
    ex = np.exp(logits - m)
    probs = ex / ex.sum(axis=-1, keepdims=True)
    order = np.argsort(-probs, axis=-1, kind="stable")[:, :TOPK]
    rows = np.arange(logits.shape[0])[:, None]
    topk_p = probs[rows, order]
    topk_p = topk_p / topk_p.sum(axis=-1, keepdims=True)
    return order, topk_p.astype(np.float32)


def kernel(x, router_logits, w1, w2):
    x = np.ascontiguousarray(np.asarray(x, dtype=np.float32))
    w1 = np.asarray(w1, dtype=np.float32)
    w2 = np.asarray(w2, dtype=np.float32)
    t = x.shape[0]

    top2_idx, top2_gate = route(router_logits)

    expert_tokens = []
    expert_gates = []
    for e in range(E):
        sel = np.nonzero(top2_idx == e)
        expert_tokens.append(sel[0])
        expert_gates.append(top2_gate[sel[0], sel[1]])
    counts = [len(ix) for ix in expert_tokens]
    # fp32r matmuls require even free/partition sizes (2-element PSUM
    # interleave), so round the computed token count up to even
    count = max(2, max(counts) + max(counts) % 2)
    C = -(-count // P) * P  # buffer capacity (128-aligned)

    nc = build_moe_expert_kernel(count)

    in_maps = []
    for e in range(E):
        cnt = counts[e]
        xT_e = np.zeros((H, C), dtype=np.float32)
        xT_e[:, :cnt] = x[expert_tokens[e]].T
        g = np.zeros((1, C), dtype=np.float32)
        g[0, :cnt] = expert_gates[e]
        in_maps.append(
            {
                "xT": xT_e,
                "w1T": np.ascontiguousarray(w1[e].T),
                "w2T": np.ascontiguousarray(w2[e].T),
                "gates": g,
            }
        )

    res = run_bass_kernel_spmd(nc, in_maps, core_ids=list(range(N_CORES)))
    if not all(np.isfinite(r["yT"]).all() for r in res.results):
        # one retry in case of a transient device fault
        res = run_bass_kernel_spmd(nc, in_maps, core_ids=list(range(N_CORES)))

    out = np.zeros((t, H), dtype=np.float32)
    for e in range(E):
        cnt = counts[e]
        out[expert_tokens[e]] += res.results[e]["yT"][:, :cnt].T
    return out


# revision 70
# speedup vs baseline: 1.0102x; 1.0058x over previous
"""MoE (top-2 of 8 experts) Trainium2 kernel.

Strategy: expert-parallel across the 8 NeuronCores. The router
(softmax + top-2 over [T, 8] logits) is metadata computed on host to
build the dispatch; core e receives only the tokens routed to expert e
(gathered, transposed, zero-padded to a common capacity C) plus that
expert's weights, pre-transposed so the device does no transposes:

  core e inputs:  xT  [H, C]   = x[idx_e].T        (padded)
                  w1T [H, I]   = w1[e].T
                  w2T [I, H]   = w2[e].T
                  gates [1, C]  renormalized top-2 weight per token
  core e output:  yT  [H, C]  = (gate * (silu(x_e @ w1[e].T) @ w2[e].T)).T

On device (per core, fp32 storage, float32r matmuls, only the exact
even-rounded token count is computed — no padding columns):
  stage 1: hT[i_tile, c_chunk] = silu(w1T.T @ xT)    (I on partitions)
  stage 2: yT[h_tile, c_chunk] = w2T.T @ hT, times the per-token gate
           (w2 stationary, hT moving: the stream covers the ragged token
           dim; gate is broadcast to all partitions by a 0-stride DMA)

The host transposes and scatter-adds the two expert contributions per
token.
"""

import numpy as np

import concourse.mybir as mybir
from concourse import bacc
from concourse.tile import TileContext
from concourse.bass_utils import run_bass_kernel_spmd

T, H, I, E = 4096, 1024, 1408, 8
TOPK = 2
P = 128
CHUNK = 512
N_CORES = 8
F32 = mybir.dt.float32
F32R = mybir.dt.float32r
AF = mybir.ActivationFunctionType

# most recently built device program (for test harnesses / cost-model timing)
LAST_NC = None


def _chunk_sizes(C):
    """Split C into ceil(C/512) chunks, multiples of 128, as even as
    possible. Balanced chunks keep every stage-1 matmul's moving dim >=256
    (the fp32r full-rate threshold) instead of a slow ragged tail."""
    n = -(-C // CHUNK)
    base = (C // n) // P * P
    rem = (C - n * base) // P
    return [base + P if j < rem else base for j in range(n)]


def _count_chunks(count):
    """Chunk an exact token count: every chunk starts 128-aligned, all but
    the last are multiples of 128, the last is ragged — no cycles are spent
    on padding columns. Front chunks are full 512 so stage-1 groups outlast
    the w1 tile arrival period (no DMA-pacing stalls during the weight
    stream); the tail is split to keep every chunk >=256 (the fp32r
    full-rate threshold) whenever count allows."""
    full, rem = divmod(count, CHUNK)
    if rem == 0:
        return [CHUNK] * full
    if rem >= 256 or full == 0:
        return [CHUNK] * full + [rem]
    # rem < 256: borrow from one full chunk so both tail chunks stay >=256
    return [CHUNK] * (full - 1) + [256, 256 + rem]


def build_moe_expert_kernel(count, h=H, i_dim=I):
    """One-expert MLP over `count` gathered tokens (any positive int —
    DRAM buffers are padded to a 128 multiple, but only `count` columns
    are computed). h, i_dim overridable for small-scale simulation tests;
    both must be multiples of 128. count must be even (fp32r matmuls
    reject odd free/partition sizes)."""
    C = -(-count // P) * P  # DRAM/layout capacity
    assert count % 2 == 0 and h % P == 0 and i_dim % P == 0
    HK = h // P
    IT = i_dim // P

    nc = bacc.Bacc("TRN2", target_bir_lowering=False, debug=False, num_devices=N_CORES)
    # Matmul inputs are stored as float32r (same 32-bit layout; the PE
    # rounds to its reduced internal precision). Typing the whole producer
    # chain as f32r satisfies the BIR verifier's rounding check.
    xT = nc.dram_tensor("xT", [h, C], F32R, kind="ExternalInput").ap()
    w1T = nc.dram_tensor("w1T", [h, i_dim], F32R, kind="ExternalInput").ap()
    w2T = nc.dram_tensor("w2T", [i_dim, h], F32R, kind="ExternalInput").ap()
    gates = nc.dram_tensor("gates", [1, C], F32, kind="ExternalInput").ap()
    # host-packed first-group operands: per partition p (= h row p),
    # [w1T[p, 0:128] | xT[p, 0:cs0]] — one DMA arms the first matmul
    cs0_pre = _count_chunks(count)[0]
    prelude = nc.dram_tensor("prelude", [P, P + cs0_pre], F32R, kind="ExternalInput").ap()
    # output is yT [h, C]: stage 2 streams over the ragged token dim, so
    # tokens land on the free axis (the host transposes back)
    yT = nc.dram_tensor("yT", [h, C], F32, kind="ExternalOutput").ap()

    xT_v = xT.rearrange("(ho p) c -> p ho c", p=P)  # [128, HK, C]
    w1T_v = w1T.rearrange("(ho p) i -> p ho i", p=P)  # [128, HK, I]
    w2T_v = w2T.rearrange("(io p) h -> p io h", p=P)  # [128, IT, H]
    yT_v = yT.rearrange("(ho p) c -> ho p c", p=P)  # [HK, 128, C]

    h_chunks = _chunk_sizes(h)  # h-chunks for stage 2 output
    c_chunks = _count_chunks(count)
    max_cs = max(c_chunks)
    c_starts = [sum(c_chunks[:j]) for j in range(len(c_chunks))]
    # per-partition SBUF bytes: weights + broadcast gates + h/sg bufs; give
    # the x and y pools extra bufs only while the 192 KB budget holds
    base = 4 * (HK * i_dim + IT * h + C + 2 * IT * max_cs + 2 * CHUNK)
    x_bufs = 3 if base + 3 * 4 * HK * max_cs + 2 * 4 * CHUNK < 190 * 1024 else 2
    fixed = base + x_bufs * 4 * HK * max_cs
    y_bufs = 4 if fixed + 4 * 4 * CHUNK < 190 * 1024 else 2
    with TileContext(nc) as tc:
        with (
            tc.tile_pool(name="wpool", bufs=1) as wpool,
            tc.tile_pool(name="xpool", bufs=x_bufs) as xpool,
            tc.tile_pool(name="hpool", bufs=2) as hpool,
            tc.tile_pool(name="ypool", bufs=y_bufs) as ypool,
            tc.tile_pool(name="sgpool", bufs=2) as sgpool,
            tc.tile_pool(name="ps1", bufs=4, space="PSUM") as ps1pool,
            tc.tile_pool(name="ps2", bufs=4, space="PSUM") as ps2pool,
        ):
            # per-token gate replicated to all partitions: one DMA reading
            # the same DRAM row 128x (0-stride partition source)
            gb = wpool.tile([P, C], F32)
            w1s = wpool.tile([P, HK, i_dim], F32R)
            w2s = wpool.tile([P, IT, h], F32R)
            xs_tiles = {}

            def load_x(ci, split=True):
                # per-hk DMAs deliver the chunk incrementally so stage-1
                # groups can start before the whole chunk lands
                xs = xpool.tile([P, HK, max_cs], F32R, tag="xs", name=f"xs{ci}")
                cs, c0 = c_chunks[ci], c_starts[ci]
                if split:
                    for hk in range(HK):
                        nc.sync.dma_start(xs[:, hk, :cs], xT_v[:, hk, c0 : c0 + cs])
                else:
                    nc.sync.dma_start(xs[:, :, :cs], xT_v[:, :, c0 : c0 + cs])
                xs_tiles[ci] = xs

            def load_w1(it):
                nc.sync.dma_start(
                    w1s[:, :, it * P : (it + 1) * P],
                    w1T_v[:, :, it * P : (it + 1) * P],
                )

            # DMA issue order = consumption order. Interleave chunk-0 x
            # slices with the leading w1 i-tiles so the first stage-1
            # accumulation group starts after ~0.7 MB instead of ~6 MB;
            # then the rest of w1, the remaining x chunks, then w2 (per
            # h-half, consumed by stage 2).
            xs0 = xpool.tile([P, HK, max_cs], F32R, tag="xs", name="xs0")
            cs0 = c_chunks[0]
            pre = wpool.tile([P, P + cs0], F32R)
            nc.sync.dma_start(pre[:], prelude[:])
            # w1 it0's hk0 slice lives in the prelude; load only hk1..
            nc.sync.dma_start(w1s[:, 1:, 0:P], w1T_v[:, 1:, 0:P])
            for hk in range(1, HK):
                nc.sync.dma_start(xs0[:, hk, :cs0], xT_v[:, hk, 0:cs0])
                if hk == min(2, HK - 1) and IT > 1:
                    load_w1(1)
            xs_tiles[0] = xs0
            for it in range(2, IT):
                load_w1(it)
            # w2 per h-half per i-tile: stage 2 consumes one h-chunk across
            # i-tiles in order, so fine-grained delivery unblocks each
            # accumulation group as early as possible
            h_starts = [sum(h_chunks[:j]) for j in range(len(h_chunks))]
            # only as many x chunks up front as there are pool slots — a
            # queued DMA waiting on a busy slot would head-of-line block
            # the w2 stream behind it; later chunks prefetch inside stage 1
            for ci in range(1, min(x_bufs, len(c_chunks))):
                load_x(ci)
            # broadcast-gate load sits after the stage-1 streams (it is
            # only needed when the first stage-2 group's psum is evacuated)
            nc.sync.dma_start(gb[:], gates[0].partition_broadcast(P))
            for h0, hcs in zip(h_starts, h_chunks):
                for it in range(IT):
                    nc.sync.dma_start(
                        w2s[:, it, h0 : h0 + hcs], w2T_v[:, it, h0 : h0 + hcs]
                    )

            hs_tiles = {}

            def stage1(ci):
                nxt = ci + 1
                if nxt < len(c_chunks) and nxt not in xs_tiles:
                    load_x(nxt)
                cs = c_chunks[ci]
                xs = xs_tiles[ci]
                # hT = silu(w1T.T @ xT)  -> [I, cs], I on partitions
                hs = hpool.tile([P, IT, max_cs], F32R, tag="hs", name=f"hs{ci}")
                for it in range(IT):
                    ps1 = ps1pool.tile([P, CHUNK], F32, tag="ps1")
                    for hk in range(HK):
                        # (it0, hk0) weights and chunk-0's hk0 x-slice live
                        # in the prelude tile (w1s[:, 0, 0:P] is never DMA'd)
                        if hk == 0 and it == 0:
                            lhsT = pre[:, 0:P]
                        else:
                            lhsT = w1s[:, hk, it * P : (it + 1) * P]
                        if ci == 0 and hk == 0:
                            rhs = pre[:, P : P + cs]
                        else:
                            rhs = xs[:, hk, :cs]
                        nc.tensor.matmul(
                            ps1[:, :cs],
                            lhsT,
                            rhs,
                            start=(hk == 0),
                            stop=(hk == HK - 1),
                        )
                    # silu(z) = z * sigmoid(z); CoreSim has no Silu table,
                    # so build it from Sigmoid (ACT) + multiply (DVE)
                    sg = sgpool.tile([P, CHUNK], F32, tag="sg")
                    nc.scalar.activation(sg[:, :cs], ps1[:, :cs], AF.Sigmoid)
                    nc.vector.tensor_mul(
                        out=hs[:, it, :cs], in0=ps1[:, :cs], in1=sg[:, :cs]
                    )
                hs_tiles[ci] = hs

            def stage2(ci):
                # yT = (w2T.T @ hT) * gate -> [H, cs], h on partitions.
                # w2 is the stationary operand and hT the moving one, so the
                # stream covers exactly the ragged token count — no padded
                # columns and no partial-partition tiles.
                cs, c0 = c_chunks[ci], c_starts[ci]
                hs = hs_tiles.pop(ci)
                for ht in range(HK):
                    ps2 = ps2pool.tile([P, CHUNK], F32, tag="ps2")
                    for it in range(IT):
                        nc.tensor.matmul(
                            ps2[:, :cs],
                            w2s[:, it, ht * P : (ht + 1) * P],
                            hs[:, it, :cs],
                            start=(it == 0),
                            stop=(it == IT - 1),
                        )
                    ys = ypool.tile([P, CHUNK], F32, tag="ys")
                    nc.vector.tensor_mul(
                        out=ys[:, :cs], in0=ps2[:, :cs], in1=gb[:, c0 : c0 + cs]
                    )
                    nc.sync.dma_start(yT_v[ht][:, c0 : c0 + cs], ys[:, :cs])

            # software pipeline: run stage 1 a chunk ahead so the PE has
            # stage-1 work for chunk i+1 while w2 is still streaming in
            stage1(0)
            for ci in range(1, len(c_chunks)):
                stage1(ci)
                stage2(ci - 1)
            stage2(len(c_chunks) - 1)
    nc.compile()
    global LAST_NC
    LAST_NC = nc
    return nc


def route(router_logits):
    """Host-side router: softmax -> top-2 -> renormalize.

    Returns (top2_idx [T,2] int64, top2_gate [T,2] float32)."""
    logits = np.asarray(router_logits, dtype=np.float32)
    m = logits.max(axis=-1, keepdims=True)
    ex = np.exp(logits - m)
    probs = ex / ex.sum(axis=-1, keepdims=True)
    order = np.argsort(-probs, axis=-1, kind="stable")[:, :TOPK]
    rows = np.arange(logits.shape[0])[:, None]
    topk_p = probs[rows, order]
    topk_p = topk_p / topk_p.sum(axis=-1, keepdims=True)
    return order, topk_p.astype(np.float32)


def kernel(x, router_logits, w1, w2):
    x = np.ascontiguousarray(np.asarray(x, dtype=np.float32))
    w1 = np.asarray(w1, dtype=np.float32)
    w2 = np.asarray(w2, dtype=np.float32)
    t = x.shape[0]

    top2_idx, top2_gate = route(router_logits)

    expert_tokens = []
    expert_gates = []
    for e in range(E):
        sel = np.nonzero(top2_idx == e)
        expert_tokens.append(sel[0])
        expert_gates.append(top2_gate[sel[0], sel[1]])
    counts = [len(ix) for ix in expert_tokens]
    # fp32r matmuls require even free/partition sizes (2-element PSUM
    # interleave), so round the computed token count up to even
    count = max(2, max(counts) + max(counts) % 2)
    C = -(-count // P) * P  # buffer capacity (128-aligned)

    nc = build_moe_expert_kernel(count)
    kernel_cs0 = _count_chunks(count)[0]

    in_maps = []
    for e in range(E):
        cnt = counts[e]
        xT_e = np.zeros((H, C), dtype=np.float32)
        xT_e[:, :cnt] = x[expert_tokens[e]].T
        g = np.zeros((1, C), dtype=np.float32)
        g[0, :cnt] = expert_gates[e]
        w1T_e = np.ascontiguousarray(w1[e].T)
        cs0 = kernel_cs0
        in_maps.append(
            {
                "xT": xT_e,
                "w1T": w1T_e,
                "w2T": np.ascontiguousarray(w2[e].T),
                "gates": g,
                "prelude": np.ascontiguousarray(
                    np.concatenate([w1T_e[:P, :P], xT_e[:P, :cs0]], axis=1)
                ),
            }
        )

    res = run_bass_kernel_spmd(nc, in_maps, core_ids=list(range(N_CORES)))
    if not all(np.isfinite(r["yT"]).all() for r in res.results):
        # one retry in case of a transient device fault
        res = run_bass_kernel_spmd(nc, in_maps, core_ids=list(range(N_CORES)))

    out = np.zeros((t, H), dtype=np.float32)
    for e in range(E):
        cnt = counts[e]
        out[expert_tokens[e]] += res.results[e]["yT"][:, :cnt].T
    return out


# revision 71
# speedup vs baseline: 1.0110x; 1.0008x over previous
"""MoE (top-2 of 8 experts) Trainium2 kernel.

Strategy: expert-parallel across the 8 NeuronCores. The router
(softmax + top-2 over [T, 8] logits) is metadata computed on host to
build the dispatch; core e receives only the tokens routed to expert e
(gathered, transposed, zero-padded to a common capacity C) plus that
expert's weights, pre-transposed so the device does no transposes:

  core e inputs:  xT  [H, C]   = x[idx_e].T        (padded)
                  w1T [H, I]   = w1[e].T
                  w2T [I, H]   = w2[e].T
                  gates [1, C]  renormalized top-2 weight per token
  core e output:  yT  [H, C]  = (gate * (silu(x_e @ w1[e].T) @ w2[e].T)).T

On device (per core, fp32 storage, float32r matmuls, only the exact
even-rounded token count is computed — no padding columns):
  stage 1: hT[i_tile, c_chunk] = silu(w1T.T @ xT)    (I on partitions)
  stage 2: yT[h_tile, c_chunk] = w2T.T @ hT, times the per-token gate
           (w2 stationary, hT moving: the stream covers the ragged token
           dim; gate is broadcast to all partitions by a 0-stride DMA)

The host transposes and scatter-adds the two expert contributions per
token.
"""

import numpy as np

import concourse.mybir as mybir
from concourse import bacc
from concourse.tile import TileContext
from concourse.bass_utils import run_bass_kernel_spmd

T, H, I, E = 4096, 1024, 1408, 8
TOPK = 2
P = 128
CHUNK = 512
N_CORES = 8
F32 = mybir.dt.float32
F32R = mybir.dt.float32r
AF = mybir.ActivationFunctionType

# most recently built device program (for test harnesses / cost-model timing)
LAST_NC = None


def _chunk_sizes(C):
    """Split C into ceil(C/512) chunks, multiples of 128, as even as
    possible. Balanced chunks keep every stage-1 matmul's moving dim >=256
    (the fp32r full-rate threshold) instead of a slow ragged tail."""
    n = -(-C // CHUNK)
    base = (C // n) // P * P
    rem = (C - n * base) // P
    return [base + P if j < rem else base for j in range(n)]


def _count_chunks(count):
    """Chunk an exact token count (no alignment needed: both stages
    stream the token dim). Front chunks are full 512 so stage-1 groups outlast
    the w1 tile arrival period (no DMA-pacing stalls during the weight
    stream); the tail is split to keep every chunk >=256 (the fp32r
    full-rate threshold) whenever count allows."""
    full, rem = divmod(count, CHUNK)
    if rem == 0:
        return [CHUNK] * full
    if rem >= 256 or full == 0:
        return [CHUNK] * full + [rem]
    # rem < 256: borrow from one full chunk so both tail chunks stay >=256
    return [CHUNK] * (full - 1) + [256 + rem, 256]


def build_moe_expert_kernel(count, h=H, i_dim=I):
    """One-expert MLP over `count` gathered tokens (any positive int —
    DRAM buffers are padded to a 128 multiple, but only `count` columns
    are computed). h, i_dim overridable for small-scale simulation tests;
    both must be multiples of 128. count must be even (fp32r matmuls
    reject odd free/partition sizes)."""
    C = -(-count // P) * P  # DRAM/layout capacity
    assert count % 2 == 0 and h % P == 0 and i_dim % P == 0
    HK = h // P
    IT = i_dim // P

    nc = bacc.Bacc("TRN2", target_bir_lowering=False, debug=False, num_devices=N_CORES)
    # Matmul inputs are stored as float32r (same 32-bit layout; the PE
    # rounds to its reduced internal precision). Typing the whole producer
    # chain as f32r satisfies the BIR verifier's rounding check.
    xT = nc.dram_tensor("xT", [h, C], F32R, kind="ExternalInput").ap()
    w1T = nc.dram_tensor("w1T", [h, i_dim], F32R, kind="ExternalInput").ap()
    w2T = nc.dram_tensor("w2T", [i_dim, h], F32R, kind="ExternalInput").ap()
    gates = nc.dram_tensor("gates", [1, C], F32, kind="ExternalInput").ap()
    # host-packed first-group operands: per partition p (= h row p),
    # [w1T[p, 0:128] | xT[p, 0:cs0]] — one DMA arms the first matmul
    cs0_pre = _count_chunks(count)[0]
    prelude = nc.dram_tensor("prelude", [P, P + cs0_pre], F32R, kind="ExternalInput").ap()
    # output is yT [h, C]: stage 2 streams over the ragged token dim, so
    # tokens land on the free axis (the host transposes back)
    yT = nc.dram_tensor("yT", [h, C], F32, kind="ExternalOutput").ap()

    xT_v = xT.rearrange("(ho p) c -> p ho c", p=P)  # [128, HK, C]
    w1T_v = w1T.rearrange("(ho p) i -> p ho i", p=P)  # [128, HK, I]
    w2T_v = w2T.rearrange("(io p) h -> p io h", p=P)  # [128, IT, H]
    yT_v = yT.rearrange("(ho p) c -> ho p c", p=P)  # [HK, 128, C]

    h_chunks = _chunk_sizes(h)  # h-chunks for stage 2 output
    c_chunks = _count_chunks(count)
    max_cs = max(c_chunks)
    c_starts = [sum(c_chunks[:j]) for j in range(len(c_chunks))]
    # per-partition SBUF bytes: weights + broadcast gates + h/sg bufs; give
    # the x and y pools extra bufs only while the 192 KB budget holds
    base = 4 * (HK * i_dim + IT * h + C + 2 * IT * max_cs + 2 * CHUNK)
    x_bufs = 3 if base + 3 * 4 * HK * max_cs + 2 * 4 * CHUNK < 190 * 1024 else 2
    fixed = base + x_bufs * 4 * HK * max_cs
    y_bufs = 4 if fixed + 4 * 4 * CHUNK < 190 * 1024 else 2
    with TileContext(nc) as tc:
        with (
            tc.tile_pool(name="wpool", bufs=1) as wpool,
            tc.tile_pool(name="xpool", bufs=x_bufs) as xpool,
            tc.tile_pool(name="hpool", bufs=2) as hpool,
            tc.tile_pool(name="ypool", bufs=y_bufs) as ypool,
            tc.tile_pool(name="sgpool", bufs=2) as sgpool,
            tc.tile_pool(name="ps1", bufs=4, space="PSUM") as ps1pool,
            tc.tile_pool(name="ps2", bufs=4, space="PSUM") as ps2pool,
        ):
            # per-token gate replicated to all partitions: one DMA reading
            # the same DRAM row 128x (0-stride partition source)
            gb = wpool.tile([P, C], F32)
            w1s = wpool.tile([P, HK, i_dim], F32R)
            w2s = wpool.tile([P, IT, h], F32R)
            xs_tiles = {}

            def load_x(ci, split=True):
                # per-hk DMAs deliver the chunk incrementally so stage-1
                # groups can start before the whole chunk lands
                xs = xpool.tile([P, HK, max_cs], F32R, tag="xs", name=f"xs{ci}")
                cs, c0 = c_chunks[ci], c_starts[ci]
                if split:
                    for hk in range(HK):
                        nc.sync.dma_start(xs[:, hk, :cs], xT_v[:, hk, c0 : c0 + cs])
                else:
                    nc.sync.dma_start(xs[:, :, :cs], xT_v[:, :, c0 : c0 + cs])
                xs_tiles[ci] = xs

            def load_w1(it):
                nc.sync.dma_start(
                    w1s[:, :, it * P : (it + 1) * P],
                    w1T_v[:, :, it * P : (it + 1) * P],
                )

            # DMA issue order = consumption order. Interleave chunk-0 x
            # slices with the leading w1 i-tiles so the first stage-1
            # accumulation group starts after ~0.7 MB instead of ~6 MB;
            # then the rest of w1, the remaining x chunks, then w2 (per
            # h-half, consumed by stage 2).
            xs0 = xpool.tile([P, HK, max_cs], F32R, tag="xs", name="xs0")
            cs0 = c_chunks[0]
            pre = wpool.tile([P, P + cs0], F32R)
            nc.sync.dma_start(pre[:], prelude[:])
            # w1 it0's hk0 slice lives in the prelude; load only hk1..
            nc.sync.dma_start(w1s[:, 1:, 0:P], w1T_v[:, 1:, 0:P])
            for hk in range(1, HK):
                nc.sync.dma_start(xs0[:, hk, :cs0], xT_v[:, hk, 0:cs0])
                if hk == min(2, HK - 1) and IT > 1:
                    load_w1(1)
            xs_tiles[0] = xs0
            for it in range(2, IT):
                load_w1(it)
            # w2 per h-half per i-tile: stage 2 consumes one h-chunk across
            # i-tiles in order, so fine-grained delivery unblocks each
            # accumulation group as early as possible
            h_starts = [sum(h_chunks[:j]) for j in range(len(h_chunks))]
            # only as many x chunks up front as there are pool slots — a
            # queued DMA waiting on a busy slot would head-of-line block
            # the w2 stream behind it; later chunks prefetch inside stage 1
            for ci in range(1, min(x_bufs, len(c_chunks))):
                load_x(ci)
            # broadcast-gate load sits after the stage-1 streams (it is
            # only needed when the first stage-2 group's psum is evacuated)
            nc.sync.dma_start(gb[:], gates[0].partition_broadcast(P))
            for h0, hcs in zip(h_starts, h_chunks):
                for it in range(IT):
                    nc.sync.dma_start(
                        w2s[:, it, h0 : h0 + hcs], w2T_v[:, it, h0 : h0 + hcs]
                    )

            hs_tiles = {}

            def stage1(ci):
                nxt = ci + 1
                if nxt < len(c_chunks) and nxt not in xs_tiles:
                    load_x(nxt)
                cs = c_chunks[ci]
                xs = xs_tiles[ci]
                # hT = silu(w1T.T @ xT)  -> [I, cs], I on partitions
                hs = hpool.tile([P, IT, max_cs], F32R, tag="hs", name=f"hs{ci}")
                for it in range(IT):
                    ps1 = ps1pool.tile([P, CHUNK], F32, tag="ps1")
                    for hk in range(HK):
                        # (it0, hk0) weights and chunk-0's hk0 x-slice live
                        # in the prelude tile (w1s[:, 0, 0:P] is never DMA'd)
                        if hk == 0 and it == 0:
                            lhsT = pre[:, 0:P]
                        else:
                            lhsT = w1s[:, hk, it * P : (it + 1) * P]
                        if ci == 0 and hk == 0:
                            rhs = pre[:, P : P + cs]
                        else:
                            rhs = xs[:, hk, :cs]
                        nc.tensor.matmul(
                            ps1[:, :cs],
                            lhsT,
                            rhs,
                            start=(hk == 0),
                            stop=(hk == HK - 1),
                        )
                    # silu(z) = z * sigmoid(z); CoreSim has no Silu table,
                    # so build it from Sigmoid (ACT) + multiply (DVE)
                    sg = sgpool.tile([P, CHUNK], F32, tag="sg")
                    nc.scalar.activation(sg[:, :cs], ps1[:, :cs], AF.Sigmoid)
                    nc.vector.tensor_mul(
                        out=hs[:, it, :cs], in0=ps1[:, :cs], in1=sg[:, :cs]
                    )
                hs_tiles[ci] = hs

            def stage2(ci):
                # yT = (w2T.T @ hT) * gate -> [H, cs], h on partitions.
                # w2 is the stationary operand and hT the moving one, so the
                # stream covers exactly the ragged token count — no padded
                # columns and no partial-partition tiles.
                cs, c0 = c_chunks[ci], c_starts[ci]
                hs = hs_tiles.pop(ci)
                for ht in range(HK):
                    ps2 = ps2pool.tile([P, CHUNK], F32, tag="ps2")
                    for it in range(IT):
                        nc.tensor.matmul(
                            ps2[:, :cs],
                            w2s[:, it, ht * P : (ht + 1) * P],
                            hs[:, it, :cs],
                            start=(it == 0),
                            stop=(it == IT - 1),
                        )
                    ys = ypool.tile([P, CHUNK], F32, tag="ys")
                    nc.vector.tensor_mul(
                        out=ys[:, :cs], in0=ps2[:, :cs], in1=gb[:, c0 : c0 + cs]
                    )
                    nc.sync.dma_start(yT_v[ht][:, c0 : c0 + cs], ys[:, :cs])

            # software pipeline: run stage 1 a chunk ahead so the PE has
            # stage-1 work for chunk i+1 while w2 is still streaming in
            stage1(0)
            for ci in range(1, len(c_chunks)):
                stage1(ci)
                stage2(ci - 1)
            stage2(len(c_chunks) - 1)
    nc.compile()
    global LAST_NC
    LAST_NC = nc
    return nc


def route(router_logits):
    """Host-side router: softmax -> top-2 -> renormalize.

    Returns (top2_idx [T,2] int64, top2_gate [T,2] float32)."""
    logits = np.asarray(router_logits, dtype=np.float32)
    m = logits.max(axis=-1, keepdims=True)
    ex = np.exp(logits - m)
    probs = ex / ex.sum(axis=-1, keepdims=True)
    order = np.argsort(-probs, axis=-1, kind="stable")[:, :TOPK]
    rows = np.arange(logits.shape[0])[:, None]
    topk_p = probs[rows, order]
    topk_p = topk_p / topk_p.sum(axis=-1, keepdims=True)
    return order, topk_p.astype(np.float32)


def kernel(x, router_logits, w1, w2):
    x = np.ascontiguousarray(np.asarray(x, dtype=np.float32))
    w1 = np.asarray(w1, dtype=np.float32)
    w2 = np.asarray(w2, dtype=np.float32)
    t = x.shape[0]

    top2_idx, top2_gate = route(router_logits)

    expert_tokens = []
    expert_gates = []
    for e in range(E):
        sel = np.nonzero(top2_idx == e)
        expert_tokens.append(sel[0])
        expert_gates.append(top2_gate[sel[0], sel[1]])
    counts = [len(ix) for ix in expert_tokens]
    # fp32r matmuls require even free/partition sizes (2-element PSUM
    # interleave), so round the computed token count up to even
    count = max(2, max(counts) + max(counts) % 2)
    C = -(-count // P) * P  # buffer capacity (128-aligned)

    nc = build_moe_expert_kernel(count)
    kernel_cs0 = _count_chunks(count)[0]

    in_maps = []
    for e in range(E):
        cnt = counts[e]
        xT_e = np.zeros((H, C), dtype=np.float32)
        xT_e[:, :cnt] = x[expert_tokens[e]].T
        g = np.zeros((1, C), dtype=np.float32)
        g[0, :cnt] = expert_gates[e]
        w1T_e = np.ascontiguousarray(w1[e].T)
        cs0 = kernel_cs0
        in_maps.append(
            {
                "xT": xT_e,
                "w1T": w1T_e,
                "w2T": np.ascontiguousarray(w2[e].T),
                "gates": g,
                "prelude": np.ascontiguousarray(
                    np.concatenate([w1T_e[:P, :P], xT_e[:P, :cs0]], axis=1)
                ),
            }
        )

    res = run_bass_kernel_spmd(nc, in_maps, core_ids=list(range(N_CORES)))
    if not all(np.isfinite(r["yT"]).all() for r in res.results):
        # one retry in case of a transient device fault
        res = run_bass_kernel_spmd(nc, in_maps, core_ids=list(range(N_CORES)))

    out = np.zeros((t, H), dtype=np.float32)
    for e in range(E):
        cnt = counts[e]
        out[expert_tokens[e]] += res.results[e]["yT"][:, :cnt].T
    return out
